# revision 16
# baseline (speedup 1.0000x reference)
"""Trainium2 Bass kernel for nn_CalculateAttention (B=2, H=16, S=2048, D=64, fp32).

Strategy: shard the 32 (batch*head) attention instances across 8 cores (4 per
core); each core computes full attention for its heads independently.

Per-head math on device (S^T formulation so softmax's reduction lands on the
matmul contraction axis instead of the partition axis):
  - MM1:  S^T[k, q] = matmul(lhsT=K^T[d, k-tile], rhs=Q^T[d, q-chunk]), fp32r
  - ACT:  E = exp(S^T / sqrt(D))  (scale fused into ACTIVATE's affine pre-op; no
          max-subtraction needed: |scores| <= ~6 for N(0,1) inputs, fp32-safe)
  - MM2:  matmul(lhsT=V''[k-tile, 0:65], rhs=E) accumulated over k-tiles, where
          V'' = [V | ones] (ones appended host-side); accumulator row 64 is the
          softmax denominator.
  - Normalize: recip(denom), partition-broadcast via a DRAM-bounce DMA,
          multiply, DMA out as O^T[d, q].
Host side only reshapes/transposes (layout prep + unshard).
"""

import numpy as np

_B, _H, _S, _D = 2, 16, 2048, 64
_NCORES = 8
_HPC = (_B * _H) // _NCORES  # heads per core
_QCHUNK = 1024  # q columns processed per PSUM S^T tile (2 banks)
_KTILE = 128  # k rows per S^T tile (partition dim)
_MMN = 512  # max fp32 matmul free dim

_nc_cache = None


def _build_nc(hpc=_HPC, s=_S, d=_D, qchunk=_QCHUNK, reps=1):
    import concourse.bacc as bacc
    import concourse.tile as tile
    from concourse import mybir

    fp32 = mybir.dt.float32
    fp32r = mybir.dt.float32r
    n_k = s // _KTILE
    n_qc = s // qchunk
    mmn = min(_MMN, qchunk)
    n_mm = qchunk // mmn
    scale = 1.0 / float(np.sqrt(np.float32(d)))

    nc = bacc.Bacc("TRN2")
    QT = nc.dram_tensor("QT", [hpc, d, s], fp32r, kind="ExternalInput")
    KT = nc.dram_tensor("KT", [hpc, d, s], fp32r, kind="ExternalInput")
    # V'' = [V | ones], host-prepared in [k%128, k//128, d+1] layout
    V = nc.dram_tensor("V", [hpc, _KTILE, n_k, d + 1], fp32r, kind="ExternalInput")
    OT = nc.dram_tensor("OT", [hpc, d, s], fp32, kind="ExternalOutput")

    with tile.TileContext(nc) as tc:
        with (
            tc.tile_pool(name="qk", bufs=2) as qk_pool,
            tc.tile_pool(name="vp", bufs=2) as v_pool,
            tc.tile_pool(name="exp", bufs=3) as exp_pool,
            tc.tile_pool(name="outp", bufs=2) as out_pool,
            tc.tile_pool(name="small", bufs=2) as small_pool,
            tc.tile_pool(name="ps_s", bufs=2, space="PSUM") as ps_s,
            tc.tile_pool(name="ps_acc", bufs=2, space="PSUM") as ps_acc,
            tc.tile_pool(name="dram", bufs=2, space="DRAM") as dram_pool,
        ):

            def emit_body():
                for h in range(hpc):
                    qt = qk_pool.tile([d, s], fp32r, tag="qt")
                    kt = qk_pool.tile([d, s], fp32r, tag="kt")
                    nc.sync.dma_start(out=qt, in_=QT[h])
                    nc.sync.dma_start(out=kt, in_=KT[h])
                    vpp = v_pool.tile([_KTILE, n_k, d + 1], fp32r, tag="v")
                    nc.sync.dma_start(out=vpp, in_=V[h])

                    for qc in range(n_qc):
                        q0 = qc * qchunk
                        acc = ps_acc.tile([d + 1, qchunk], fp32, tag="acc")
                        for k in range(n_k):
                            st = ps_s.tile([_KTILE, qchunk], fp32, tag="st")
                            for j in range(n_mm):
                                nc.tensor.matmul(
                                    st[:, j * mmn : (j + 1) * mmn],
                                    lhsT=kt[:, k * _KTILE : (k + 1) * _KTILE],
                                    rhs=qt[:, q0 + j * mmn : q0 + (j + 1) * mmn],
                                    start=True,
                                    stop=True,
                                )
                            ex = exp_pool.tile([_KTILE, qchunk], fp32r, tag="ex")
                            nc.scalar.activation(
                                out=ex,
                                in_=st,
                                func=mybir.ActivationFunctionType.Exp,
                                scale=scale,
                            )
                            for j in range(n_mm):
                                nc.tensor.matmul(
                                    acc[:, j * mmn : (j + 1) * mmn],
                                    lhsT=vpp[:, k, :],
                                    rhs=ex[:, j * mmn : (j + 1) * mmn],
                                    start=(k == 0),
                                    stop=(k == n_k - 1),
                                )
                        # normalize: out[d, q] = acc[d, q] * (1 / acc[64, q])
                        rec = small_pool.tile([1, qchunk], fp32, tag="rec")
                        nc.vector.reciprocal(out=rec, in_=acc[d : d + 1, :])
                        # replicate recip row across d partitions: bounce
                        # through DRAM (DRAM-source DMA allows stride-0 reads)
                        dscr = dram_pool.tile([1, qchunk], fp32, tag="dscr")
                        nc.sync.dma_start(out=dscr, in_=rec)
                        bcs = small_pool.tile([d, qchunk], fp32, tag="bc")
                        nc.gpsimd.dma_start(
                            out=bcs, in_=dscr.to_broadcast((d, qchunk))
                        )
                        ob = out_pool.tile([d, qchunk], fp32, tag="ob")
                        nc.vector.tensor_mul(ob, acc[0:d, :], bcs)
                        nc.sync.dma_start(out=OT[h, :, q0 : q0 + qchunk], in_=ob)

            if reps == 1:
                emit_body()
            else:
                with tc.For_i(0, reps, 1):
                    emit_body()
    nc.compile()
    return nc


def _shard_inputs(Q, K, V):
    """Full [B,H,S,D] inputs -> per-core in_maps with pre-transposed Q/K and
    ones-augmented, DMA-friendly V layout."""
    bh = _B * _H
    n_k = _S // _KTILE
    Qf = np.ascontiguousarray(
        np.asarray(Q, dtype=np.float32).reshape(bh, _S, _D).transpose(0, 2, 1)
    )
    Kf = np.ascontiguousarray(
        np.asarray(K, dtype=np.float32).reshape(bh, _S, _D).transpose(0, 2, 1)
    )
    Vf = np.asarray(V, dtype=np.float32).reshape(bh, _S, _D)
    Vf = np.concatenate([Vf, np.ones((bh, _S, 1), np.float32)], axis=2)
    # [bh, S, D+1] -> [bh, k%128, k//128, D+1]
    Vf = np.ascontiguousarray(
        Vf.reshape(bh, n_k, _KTILE, _D + 1).transpose(0, 2, 1, 3)
    )
    in_maps = []
    for c in range(_NCORES):
        lo, hi = c * _HPC, (c + 1) * _HPC
        in_maps.append({"QT": Qf[lo:hi], "KT": Kf[lo:hi], "V": Vf[lo:hi]})
    return in_maps


def _unshard_output(results):
    ot = np.concatenate([r["OT"] for r in results], axis=0)  # [32, 64, 2048]
    return np.ascontiguousarray(
        ot.transpose(0, 2, 1).reshape(_B, _H, _S, _D).astype(np.float32)
    )


def kernel(Q, K, V):
    global _nc_cache
    from concourse import bass_utils

    if _nc_cache is None:
        _nc_cache = _build_nc()
    in_maps = _shard_inputs(Q, K, V)
    res = bass_utils.run_bass_kernel_spmd(
        _nc_cache, in_maps, core_ids=list(range(_NCORES))
    )
    return _unshard_output(res.results)


# revision 23
# speedup vs baseline: 1.3462x; 1.3462x over previous
"""Trainium2 Bass kernel for nn_CalculateAttention (B=2, H=16, S=2048, D=64, fp32).

Strategy: shard the 32 (batch*head) attention instances across 8 cores (4 per
core); each core computes full attention for its heads independently, two
heads interleaved through the pipeline at a time.

Per-head math on device (S^T formulation so softmax's reduction lands on the
matmul contraction axis instead of the partition axis):
  - MM1:  S^T[k, q] = matmul(lhsT=K^T[d, k-tile], rhs=Q^T[d, q-chunk]), fp32r.
          The two heads of a pair are stacked on partitions 0-63 / 64-127, so
          their K=64 matmuls occupy disjoint PE row-groups and run
          concurrently (row packing).
  - ACT:  E = exp(S^T / sqrt(D))  (scale fused into ACTIVATE's affine pre-op;
          no max-subtraction needed: |scores| <= ~6 for N(0,1) inputs).
  - MM2:  matmul(lhsT=V''[k-tile, 0:65], rhs=E) accumulated over k-tiles in
          PSUM, where V'' = [V | ones] (ones appended host-side); accumulator
          row 64 is the softmax denominator.
  - Epilogue: evacuate accumulator to SBUF (frees PSUM immediately), then
          recip(denom), partition-broadcast via a DRAM-bounce DMA, multiply,
          DMA out as O^T[d, q].
Host side only reshapes/transposes (layout prep + unshard).
"""

import numpy as np

_B, _H, _S, _D = 2, 16, 2048, 64
_NCORES = 8
_HPC = (_B * _H) // _NCORES  # heads per core
_QCHUNK = 1024  # q columns per PSUM S^T tile (2 banks)
_KTILE = 128  # k rows per S^T tile (partition dim)
_MMN = 512  # max fp32 matmul free dim

_nc_cache = None


def _build_nc(hpc=_HPC, s=_S, d=_D, qchunk=_QCHUNK, reps=1, mode="full"):
    import concourse.bacc as bacc
    import concourse.tile as tile
    from concourse import mybir

    assert hpc % 2 == 0, "heads processed in pairs"
    fp32 = mybir.dt.float32
    fp32r = mybir.dt.float32r
    n_k = s // _KTILE
    n_qc = s // qchunk
    mmn = min(_MMN, qchunk)
    n_mm = qchunk // mmn
    scale = 1.0 / float(np.sqrt(np.float32(d)))

    nc = bacc.Bacc("TRN2")
    # Q^T/K^T with head pairs stacked along the partition dim: [pair, 2*d, s]
    QT = nc.dram_tensor("QT", [hpc // 2, 2 * d, s], fp32r, kind="ExternalInput")
    KT = nc.dram_tensor("KT", [hpc // 2, 2 * d, s], fp32r, kind="ExternalInput")
    # V'' = [V | ones], host-prepared in [head, k%128, k//128, d+1] layout
    V = nc.dram_tensor("V", [hpc, _KTILE, n_k, d + 1], fp32r, kind="ExternalInput")
    OT = nc.dram_tensor("OT", [hpc, d, s], fp32, kind="ExternalOutput")

    with tile.TileContext(nc) as tc:
        with (
            tc.tile_pool(name="qk", bufs=2) as qk_pool,
            tc.tile_pool(name="vp", bufs=3) as v_pool,
            tc.tile_pool(name="exp", bufs=4) as exp_pool,
            tc.tile_pool(name="acsb", bufs=4) as acsb_pool,
            tc.tile_pool(name="outp", bufs=2) as out_pool,
            tc.tile_pool(name="small", bufs=2) as small_pool,
            tc.tile_pool(name="ps_s", bufs=2, space="PSUM") as ps_s,
            tc.tile_pool(name="ps_acc", bufs=1, space="PSUM") as ps_acc,
            tc.tile_pool(name="dram", bufs=4, space="DRAM") as dram_pool,
        ):

            def epilogue(acc, h, q0):
                # evacuate PSUM accumulator to SBUF, then normalize from SBUF
                acc_sb = acsb_pool.tile([d + 1, qchunk], fp32, tag="acsb")
                nc.vector.tensor_copy(acc_sb, acc)
                rec = small_pool.tile([1, qchunk], fp32, tag="rec")
                nc.vector.reciprocal(out=rec, in_=acc_sb[d : d + 1, :])
                # replicate recip row across d partitions: bounce through
                # DRAM (DRAM-source DMA allows partition-stride-0 reads)
                dscr = dram_pool.tile([1, qchunk], fp32, tag="dscr")
                nc.sync.dma_start(out=dscr, in_=rec)
                bcs = small_pool.tile([d, qchunk], fp32, tag="bc")
                nc.gpsimd.dma_start(out=bcs, in_=dscr.to_broadcast((d, qchunk)))
                ob = out_pool.tile([d, qchunk], fp32, tag="ob")
                nc.vector.tensor_mul(ob, acc_sb[0:d, :], bcs)
                nc.sync.dma_start(out=OT[h, :, q0 : q0 + qchunk], in_=ob)

            def emit_body():
                for pair in range(hpc // 2):
                    h0, h1 = 2 * pair, 2 * pair + 1
                    qt = qk_pool.tile([2 * d, s], fp32r, tag="qt")
                    kt = qk_pool.tile([2 * d, s], fp32r, tag="kt")
                    nc.sync.dma_start(out=qt, in_=QT[pair])
                    nc.sync.dma_start(out=kt, in_=KT[pair])
                    vpp0 = v_pool.tile([_KTILE, n_k, d + 1], fp32r, tag="v")
                    vpp1 = v_pool.tile([_KTILE, n_k, d + 1], fp32r, tag="v")
                    nc.sync.dma_start(out=vpp0, in_=V[h0])
                    nc.sync.dma_start(out=vpp1, in_=V[h1])
                    if mode == "dma":
                        continue

                    for qc in range(n_qc):
                        q0 = qc * qchunk
                        acc0 = acc1 = None
                        if mode == "full":
                            acc0 = ps_acc.tile([d + 1, qchunk], fp32, tag="acc0")
                            acc1 = ps_acc.tile([d + 1, qchunk], fp32, tag="acc1")
                        for k in range(n_k):
                            k0 = k * _KTILE
                            st0 = ps_s.tile([_KTILE, qchunk], fp32, tag="st")
                            st1 = ps_s.tile([_KTILE, qchunk], fp32, tag="st")
                            # row-packed MM1s: head0 on partitions 0-63,
                            # head1 on 64-127 -> disjoint PE row groups
                            for j in range(n_mm):
                                js = slice(j * mmn, (j + 1) * mmn)
                                qs = slice(q0 + j * mmn, q0 + (j + 1) * mmn)
                                nc.tensor.matmul(
                                    st0[:, js],
                                    lhsT=kt[0:d, k0 : k0 + _KTILE],
                                    rhs=qt[0:d, qs],
                                    start=True,
                                    stop=True,
                                )
                                nc.tensor.matmul(
                                    st1[:, js],
                                    lhsT=kt[d : 2 * d, k0 : k0 + _KTILE],
                                    rhs=qt[d : 2 * d, qs],
                                    start=True,
                                    stop=True,
                                )
                            if mode == "mm1":
                                continue
                            ex0 = exp_pool.tile([_KTILE, qchunk], fp32r, tag="ex")
                            ex1 = exp_pool.tile([_KTILE, qchunk], fp32r, tag="ex")
                            nc.scalar.activation(
                                out=ex0,
                                in_=st0,
                                func=mybir.ActivationFunctionType.Exp,
                                scale=scale,
                            )
                            nc.scalar.activation(
                                out=ex1,
                                in_=st1,
                                func=mybir.ActivationFunctionType.Exp,
                                scale=scale,
                            )
                            if mode == "mm1act":
                                continue
                            for j in range(n_mm):
                                js = slice(j * mmn, (j + 1) * mmn)
                                nc.tensor.matmul(
                                    acc0[:, js],
                                    lhsT=vpp0[:, k, :],
                                    rhs=ex0[:, js],
                                    start=(k == 0),
                                    stop=(k == n_k - 1),
                                )
                            for j in range(n_mm):
                                js = slice(j * mmn, (j + 1) * mmn)
                                nc.tensor.matmul(
                                    acc1[:, js],
                                    lhsT=vpp1[:, k, :],
                                    rhs=ex1[:, js],
                                    start=(k == 0),
                                    stop=(k == n_k - 1),
                                )
                        if mode != "full":
                            continue
                        epilogue(acc0, h0, q0)
                        epilogue(acc1, h1, q0)

            if reps == 1:
                emit_body()
            else:
                with tc.For_i(0, reps, 1):
                    emit_body()
    nc.compile()
    return nc


def _shard_inputs(Q, K, V):
    """Full [B,H,S,D] inputs -> per-core in_maps: pair-stacked transposed Q/K
    and ones-augmented, DMA-friendly V layout."""
    bh = _B * _H
    n_k = _S // _KTILE
    # [bh, S, D] -> [bh, D, S] -> [bh//2, 2*D, S] (head pairs stacked)
    Qf = np.ascontiguousarray(
        np.asarray(Q, dtype=np.float32)
        .reshape(bh, _S, _D)
        .transpose(0, 2, 1)
        .reshape(bh // 2, 2 * _D, _S)
    )
    Kf = np.ascontiguousarray(
        np.asarray(K, dtype=np.float32)
        .reshape(bh, _S, _D)
        .transpose(0, 2, 1)
        .reshape(bh // 2, 2 * _D, _S)
    )
    Vf = np.asarray(V, dtype=np.float32).reshape(bh, _S, _D)
    Vf = np.concatenate([Vf, np.ones((bh, _S, 1), np.float32)], axis=2)
    # [bh, S, D+1] -> [bh, k%128, k//128, D+1]
    Vf = np.ascontiguousarray(
        Vf.reshape(bh, n_k, _KTILE, _D + 1).transpose(0, 2, 1, 3)
    )
    hpc2 = _HPC // 2
    in_maps = []
    for c in range(_NCORES):
        in_maps.append(
            {
                "QT": Qf[c * hpc2 : (c + 1) * hpc2],
                "KT": Kf[c * hpc2 : (c + 1) * hpc2],
                "V": Vf[c * _HPC : (c + 1) * _HPC],
            }
        )
    return in_maps


def _unshard_output(results):
    ot = np.concatenate([r["OT"] for r in results], axis=0)  # [32, 64, 2048]
    return np.ascontiguousarray(
        ot.transpose(0, 2, 1).reshape(_B, _H, _S, _D).astype(np.float32)
    )


def kernel(Q, K, V):
    global _nc_cache
    from concourse import bass_utils

    if _nc_cache is None:
        _nc_cache = _build_nc()
    in_maps = _shard_inputs(Q, K, V)
    res = bass_utils.run_bass_kernel_spmd(
        _nc_cache, in_maps, core_ids=list(range(_NCORES))
    )
    return _unshard_output(res.results)


# revision 29
# speedup vs baseline: 1.3769x; 1.0228x over previous
"""Trainium2 Bass kernel for nn_CalculateAttention (B=2, H=16, S=2048, D=64, fp32).

Strategy: shard the 32 (batch*head) attention instances across 8 cores (4 per
core); each core computes full attention for its heads independently, two
heads interleaved through the pipeline at a time.

Per-head math on device (S^T formulation so softmax's reduction lands on the
matmul contraction axis instead of the partition axis):
  - MM1:  S^T[k, q] = matmul(lhsT=K^T[d, k-tile], rhs=Q^T[d, q-chunk]), fp32r.
          The two heads of a pair are stacked on partitions 0-63 / 64-127, so
          their K=64 matmuls occupy disjoint PE row-groups and run
          concurrently (row packing).
  - ACT:  E = exp(S^T / sqrt(D))  (scale fused into ACTIVATE's affine pre-op;
          no max-subtraction needed: |scores| <= ~6 for N(0,1) inputs).
  - MM2:  matmul(lhsT=V''[k-tile, 0:65], rhs=E) accumulated over k-tiles in
          PSUM, where V'' = [V | ones] (ones appended host-side); accumulator
          row 64 is the softmax denominator.
  - Epilogue: evacuate accumulator to SBUF (frees PSUM immediately), then
          recip(denom), partition-broadcast via a DRAM-bounce DMA, multiply,
          DMA out as O^T[d, q].
Host side only reshapes/transposes (layout prep + unshard).
"""

import numpy as np

_B, _H, _S, _D = 2, 16, 2048, 64
_NCORES = 8
_HPC = (_B * _H) // _NCORES  # heads per core
_QCHUNK = 1024  # q columns per PSUM S^T tile (2 banks)
_KTILE = 128  # k rows per S^T tile (partition dim)
_MMN = 512  # max fp32 matmul free dim

_nc_cache = None


def _build_nc(hpc=_HPC, s=_S, d=_D, qchunk=_QCHUNK, reps=1, mode="full"):
    import concourse.bacc as bacc
    import concourse.tile as tile
    from concourse import mybir

    assert hpc % 2 == 0, "heads processed in pairs"
    fp32 = mybir.dt.float32
    fp32r = mybir.dt.float32r
    n_k = s // _KTILE
    n_qc = s // qchunk
    mmn = min(_MMN, qchunk)
    n_mm = qchunk // mmn
    scale = 1.0 / float(np.sqrt(np.float32(d)))

    nc = bacc.Bacc("TRN2")
    # Q^T/K^T with head pairs stacked along the partition dim: [pair, 2*d, s]
    QT = nc.dram_tensor("QT", [hpc // 2, 2 * d, s], fp32r, kind="ExternalInput")
    KT = nc.dram_tensor("KT", [hpc // 2, 2 * d, s], fp32r, kind="ExternalInput")
    # V'' = [V | ones], host-prepared in [head, k%128, k//128, d+1] layout
    V = nc.dram_tensor("V", [hpc, _KTILE, n_k, d + 1], fp32r, kind="ExternalInput")
    OT = nc.dram_tensor("OT", [hpc, d, s], fp32, kind="ExternalOutput")

    with tile.TileContext(nc) as tc:
        with (
            tc.tile_pool(name="qk", bufs=2) as qk_pool,
            tc.tile_pool(name="vp", bufs=3) as v_pool,
            tc.tile_pool(name="exp", bufs=4) as exp_pool,
            tc.tile_pool(name="acsb", bufs=4) as acsb_pool,
            tc.tile_pool(name="outp", bufs=2) as out_pool,
            tc.tile_pool(name="small", bufs=2) as small_pool,
            tc.tile_pool(name="ps_s", bufs=2, space="PSUM") as ps_s,
            tc.tile_pool(name="ps_acc", bufs=1, space="PSUM") as ps_acc,
            tc.tile_pool(name="dram", bufs=4, space="DRAM") as dram_pool,
        ):

            def epilogue(acc, h, q0):
                # evacuate PSUM accumulator to SBUF, then normalize from SBUF
                acc_sb = acsb_pool.tile([d + 1, qchunk], fp32, tag="acsb")
                nc.vector.tensor_copy(acc_sb, acc)
                # DVE's iterative divide is ~8 cyc/elem on a single-partition
                # row; reshape the denominator to [128, q/128] via a DRAM
                # bounce so the reciprocal costs ~q/128*8 cycles instead.
                dn = dram_pool.tile([1, qchunk], fp32, tag="dn")
                nc.sync.dma_start(out=dn, in_=acc_sb[d : d + 1, :])
                denw = small_pool.tile([128, qchunk // 128], fp32, tag="denw")
                nc.sync.dma_start(
                    out=denw, in_=dn.rearrange("o (p j) -> (o p) j", p=128)
                )
                recw = small_pool.tile([128, qchunk // 128], fp32, tag="recw")
                nc.vector.reciprocal(out=recw, in_=denw)
                dscr = dram_pool.tile([1, qchunk], fp32, tag="dscr")
                nc.sync.dma_start(
                    out=dscr.rearrange("o (p j) -> (o p) j", p=128), in_=recw
                )
                # replicate recip row across d partitions: bounce through
                # DRAM (DRAM-source DMA allows partition-stride-0 reads)
                bcs = small_pool.tile([d, qchunk], fp32, tag="bc")
                nc.gpsimd.dma_start(out=bcs, in_=dscr.to_broadcast((d, qchunk)))
                ob = out_pool.tile([d, qchunk], fp32, tag="ob")
                nc.vector.tensor_mul(ob, acc_sb[0:d, :], bcs)
                nc.sync.dma_start(out=OT[h, :, q0 : q0 + qchunk], in_=ob)

            def emit_body():
                for pair in range(hpc // 2):
                    h0, h1 = 2 * pair, 2 * pair + 1
                    qt = qk_pool.tile([2 * d, s], fp32r, tag="qt")
                    kt = qk_pool.tile([2 * d, s], fp32r, tag="kt")
                    nc.sync.dma_start(out=qt, in_=QT[pair])
                    nc.sync.dma_start(out=kt, in_=KT[pair])
                    vpp0 = v_pool.tile([_KTILE, n_k, d + 1], fp32r, tag="v")
                    vpp1 = v_pool.tile([_KTILE, n_k, d + 1], fp32r, tag="v")
                    nc.sync.dma_start(out=vpp0, in_=V[h0])
                    nc.sync.dma_start(out=vpp1, in_=V[h1])
                    if mode == "dma":
                        continue

                    for qc in range(n_qc):
                        q0 = qc * qchunk
                        acc0 = acc1 = None
                        if mode in ("full", "noepi", "mm12"):
                            acc0 = ps_acc.tile([d + 1, qchunk], fp32, tag="acc0")
                            acc1 = ps_acc.tile([d + 1, qchunk], fp32, tag="acc1")
                        for k in range(n_k):
                            k0 = k * _KTILE
                            st0 = ps_s.tile([_KTILE, qchunk], fp32, tag="st")
                            st1 = ps_s.tile([_KTILE, qchunk], fp32, tag="st")
                            # row-packed MM1s: head0 on partitions 0-63,
                            # head1 on 64-127 -> disjoint PE row groups
                            for j in range(n_mm):
                                js = slice(j * mmn, (j + 1) * mmn)
                                qs = slice(q0 + j * mmn, q0 + (j + 1) * mmn)
                                nc.tensor.matmul(
                                    st0[:, js],
                                    lhsT=kt[0:d, k0 : k0 + _KTILE],
                                    rhs=qt[0:d, qs],
                                    start=True,
                                    stop=True,
                                )
                                nc.tensor.matmul(
                                    st1[:, js],
                                    lhsT=kt[d : 2 * d, k0 : k0 + _KTILE],
                                    rhs=qt[d : 2 * d, qs],
                                    start=True,
                                    stop=True,
                                )
                            if mode == "mm1":
                                continue
                            if mode == "mm12":
                                # fake exp source: PE-only pipeline measurement
                                ex0 = qt[:, q0 : q0 + qchunk]
                                ex1 = qt[:, q0 : q0 + qchunk]
                            else:
                                ex0 = exp_pool.tile([_KTILE, qchunk], fp32r, tag="ex")
                                ex1 = exp_pool.tile([_KTILE, qchunk], fp32r, tag="ex")
                                nc.scalar.activation(
                                    out=ex0,
                                    in_=st0,
                                    func=mybir.ActivationFunctionType.Exp,
                                    scale=scale,
                                )
                                nc.scalar.activation(
                                    out=ex1,
                                    in_=st1,
                                    func=mybir.ActivationFunctionType.Exp,
                                    scale=scale,
                                )
                            if mode == "mm1act":
                                continue
                            for acc_t, vpp_t, ex_t in (
                                (acc0, vpp0, ex0),
                                (acc1, vpp1, ex1),
                            ):
                                for j in range(n_mm):
                                    js = slice(j * mmn, (j + 1) * mmn)
                                    nc.tensor.matmul(
                                        acc_t[:, js],
                                        lhsT=vpp_t[:, k, :],
                                        rhs=ex_t[:, js],
                                        start=(k == 0),
                                        stop=(k == n_k - 1),
                                    )
                        if mode != "full":
                            continue
                        epilogue(acc0, h0, q0)
                        epilogue(acc1, h1, q0)

            if reps == 1:
                emit_body()
            else:
                with tc.For_i(0, reps, 1):
                    emit_body()
    nc.compile()
    return nc


def _shard_inputs(Q, K, V):
    """Full [B,H,S,D] inputs -> per-core in_maps: pair-stacked transposed Q/K
    and ones-augmented, DMA-friendly V layout."""
    bh = _B * _H
    n_k = _S // _KTILE
    # [bh, S, D] -> [bh, D, S] -> [bh//2, 2*D, S] (head pairs stacked)
    Qf = np.ascontiguousarray(
        np.asarray(Q, dtype=np.float32)
        .reshape(bh, _S, _D)
        .transpose(0, 2, 1)
        .reshape(bh // 2, 2 * _D, _S)
    )
    Kf = np.ascontiguousarray(
        np.asarray(K, dtype=np.float32)
        .reshape(bh, _S, _D)
        .transpose(0, 2, 1)
        .reshape(bh // 2, 2 * _D, _S)
    )
    Vf = np.asarray(V, dtype=np.float32).reshape(bh, _S, _D)
    Vf = np.concatenate([Vf, np.ones((bh, _S, 1), np.float32)], axis=2)
    # [bh, S, D+1] -> [bh, k%128, k//128, D+1]
    Vf = np.ascontiguousarray(
        Vf.reshape(bh, n_k, _KTILE, _D + 1).transpose(0, 2, 1, 3)
    )
    hpc2 = _HPC // 2
    in_maps = []
    for c in range(_NCORES):
        in_maps.append(
            {
                "QT": Qf[c * hpc2 : (c + 1) * hpc2],
                "KT": Kf[c * hpc2 : (c + 1) * hpc2],
                "V": Vf[c * _HPC : (c + 1) * _HPC],
            }
        )
    return in_maps


def _unshard_output(results):
    ot = np.concatenate([r["OT"] for r in results], axis=0)  # [32, 64, 2048]
    return np.ascontiguousarray(
        ot.transpose(0, 2, 1).reshape(_B, _H, _S, _D).astype(np.float32)
    )


def kernel(Q, K, V):
    global _nc_cache
    from concourse import bass_utils

    if _nc_cache is None:
        _nc_cache = _build_nc()
    in_maps = _shard_inputs(Q, K, V)
    res = bass_utils.run_bass_kernel_spmd(
        _nc_cache, in_maps, core_ids=list(range(_NCORES))
    )
    return _unshard_output(res.results)


# revision 30
# speedup vs baseline: 1.4867x; 1.0798x over previous
"""Trainium2 Bass kernel for nn_CalculateAttention (B=2, H=16, S=2048, D=64, fp32).

Strategy: shard the 32 (batch*head) attention instances across 8 cores (4 per
core); each core computes full attention for its heads independently, two
heads interleaved through the pipeline at a time.

Per-head math on device (S^T formulation so softmax's reduction lands on the
matmul contraction axis instead of the partition axis):
  - MM1:  S^T[k, q] = matmul(lhsT=K^T[d, k-tile], rhs=Q^T[d, q-chunk]), fp32r.
          The two heads of a pair are stacked on partitions 0-63 / 64-127, so
          their K=64 matmuls occupy disjoint PE row-groups and run
          concurrently (row packing).
  - ACT:  E = exp(S^T / sqrt(D))  (scale fused into ACTIVATE's affine pre-op;
          no max-subtraction needed: |scores| <= ~6 for N(0,1) inputs).
  - MM2:  matmul(lhsT=V''[k-tile, 0:65], rhs=E) accumulated over k-tiles in
          PSUM, where V'' = [V | ones] (ones appended host-side); accumulator
          row 64 is the softmax denominator.
  - Epilogue: evacuate accumulator to SBUF (frees PSUM immediately), then
          recip(denom), partition-broadcast via a DRAM-bounce DMA, multiply,
          DMA out as O^T[d, q].
Host side only reshapes/transposes (layout prep + unshard).
"""

import numpy as np

_B, _H, _S, _D = 2, 16, 2048, 64
_NCORES = 8
_HPC = (_B * _H) // _NCORES  # heads per core
_QCHUNK = 1024  # q columns per PSUM S^T tile (2 banks)
_KTILE = 128  # k rows per S^T tile (partition dim)
_MMN = 512  # max fp32 matmul free dim

_nc_cache = None


def _build_nc(hpc=_HPC, s=_S, d=_D, qchunk=_QCHUNK, reps=1, mode="full"):
    import concourse.bacc as bacc
    import concourse.tile as tile
    from concourse import mybir

    assert hpc % 2 == 0, "heads processed in pairs"
    fp32 = mybir.dt.float32
    fp32r = mybir.dt.float32r
    n_k = s // _KTILE
    n_qc = s // qchunk
    mmn = min(_MMN, qchunk)
    n_mm = qchunk // mmn
    scale = 1.0 / float(np.sqrt(np.float32(d)))

    nc = bacc.Bacc("TRN2")
    # Q^T/K^T with head pairs stacked along the partition dim: [pair, 2*d, s]
    QT = nc.dram_tensor("QT", [hpc // 2, 2 * d, s], fp32r, kind="ExternalInput")
    KT = nc.dram_tensor("KT", [hpc // 2, 2 * d, s], fp32r, kind="ExternalInput")
    # V'' = [V | ones], host-prepared in [head, k%128, k//128, d+1] layout
    V = nc.dram_tensor("V", [hpc, _KTILE, n_k, d + 1], fp32r, kind="ExternalInput")
    OT = nc.dram_tensor("OT", [hpc, d, s], fp32, kind="ExternalOutput")

    with tile.TileContext(nc) as tc:
        with (
            tc.tile_pool(name="qk", bufs=2) as qk_pool,
            tc.tile_pool(name="vp", bufs=3) as v_pool,
            tc.tile_pool(name="exp", bufs=4) as exp_pool,
            tc.tile_pool(name="acsb", bufs=4) as acsb_pool,
            tc.tile_pool(name="outp", bufs=2) as out_pool,
            tc.tile_pool(name="small", bufs=2) as small_pool,
            tc.tile_pool(name="ps_s", bufs=2, space="PSUM") as ps_s,
            tc.tile_pool(name="ps_acc", bufs=1, space="PSUM") as ps_acc,
            tc.tile_pool(name="dram", bufs=4, space="DRAM") as dram_pool,
        ):

            def epilogue(acc, h, q0):
                # evacuate PSUM accumulator to SBUF, then normalize from SBUF
                acc_sb = acsb_pool.tile([d + 1, qchunk], fp32, tag="acsb")
                nc.vector.tensor_copy(acc_sb, acc)
                # DVE's iterative divide is ~8 cyc/elem on a single-partition
                # row; reshape the denominator to [128, q/128] via a DRAM
                # bounce so the reciprocal costs ~q/128*8 cycles instead.
                dn = dram_pool.tile([1, qchunk], fp32, tag="dn")
                nc.sync.dma_start(out=dn, in_=acc_sb[d : d + 1, :])
                denw = small_pool.tile([128, qchunk // 128], fp32, tag="denw")
                nc.sync.dma_start(
                    out=denw, in_=dn.rearrange("o (p j) -> (o p) j", p=128)
                )
                recw = small_pool.tile([128, qchunk // 128], fp32, tag="recw")
                nc.vector.reciprocal(out=recw, in_=denw)
                dscr = dram_pool.tile([1, qchunk], fp32, tag="dscr")
                nc.sync.dma_start(
                    out=dscr.rearrange("o (p j) -> (o p) j", p=128), in_=recw
                )
                # replicate recip row across d partitions: bounce through
                # DRAM (DRAM-source DMA allows partition-stride-0 reads)
                bcs = small_pool.tile([d, qchunk], fp32, tag="bc")
                nc.gpsimd.dma_start(out=bcs, in_=dscr.to_broadcast((d, qchunk)))
                ob = out_pool.tile([d, qchunk], fp32, tag="ob")
                nc.vector.tensor_mul(ob, acc_sb[0:d, :], bcs)
                nc.sync.dma_start(out=OT[h, :, q0 : q0 + qchunk], in_=ob)

            def emit_body():
                for pair in range(hpc // 2):
                    h0, h1 = 2 * pair, 2 * pair + 1
                    qt = qk_pool.tile([2 * d, s], fp32r, tag="qt")
                    kt = qk_pool.tile([2 * d, s], fp32r, tag="kt")
                    nc.sync.dma_start(out=qt, in_=QT[pair])
                    nc.sync.dma_start(out=kt, in_=KT[pair])
                    vpp0 = v_pool.tile([_KTILE, n_k, d + 1], fp32r, tag="v")
                    vpp1 = v_pool.tile([_KTILE, n_k, d + 1], fp32r, tag="v")
                    nc.sync.dma_start(out=vpp0, in_=V[h0])
                    nc.sync.dma_start(out=vpp1, in_=V[h1])
                    if mode == "dma":
                        continue

                    for qc in range(n_qc):
                        q0 = qc * qchunk
                        acc0 = acc1 = None
                        if mode in ("full", "noepi", "mm12"):
                            acc0 = ps_acc.tile([d + 1, qchunk], fp32, tag="acc0")
                            acc1 = ps_acc.tile([d + 1, qchunk], fp32, tag="acc1")

                        def emit_mm1_act(k):
                            k0 = k * _KTILE
                            st0 = ps_s.tile([_KTILE, qchunk], fp32, tag="st")
                            st1 = ps_s.tile([_KTILE, qchunk], fp32, tag="st")
                            # row-packed MM1s: head0 on partitions 0-63,
                            # head1 on 64-127 -> disjoint PE row groups
                            for j in range(n_mm):
                                js = slice(j * mmn, (j + 1) * mmn)
                                qs = slice(q0 + j * mmn, q0 + (j + 1) * mmn)
                                nc.tensor.matmul(
                                    st0[:, js],
                                    lhsT=kt[0:d, k0 : k0 + _KTILE],
                                    rhs=qt[0:d, qs],
                                    start=True,
                                    stop=True,
                                )
                                nc.tensor.matmul(
                                    st1[:, js],
                                    lhsT=kt[d : 2 * d, k0 : k0 + _KTILE],
                                    rhs=qt[d : 2 * d, qs],
                                    start=True,
                                    stop=True,
                                )
                            if mode == "mm1":
                                return None, None
                            if mode == "mm12":
                                # fake exp source: PE-only measurement
                                return (
                                    qt[:, q0 : q0 + qchunk],
                                    qt[:, q0 : q0 + qchunk],
                                )
                            ex0 = exp_pool.tile([_KTILE, qchunk], fp32r, tag="ex")
                            ex1 = exp_pool.tile([_KTILE, qchunk], fp32r, tag="ex")
                            nc.scalar.activation(
                                out=ex0,
                                in_=st0,
                                func=mybir.ActivationFunctionType.Exp,
                                scale=scale,
                            )
                            nc.scalar.activation(
                                out=ex1,
                                in_=st1,
                                func=mybir.ActivationFunctionType.Exp,
                                scale=scale,
                            )
                            return ex0, ex1

                        def emit_mm2(k, ex0, ex1):
                            for acc_t, vpp_t, ex_t in (
                                (acc0, vpp0, ex0),
                                (acc1, vpp1, ex1),
                            ):
                                for j in range(n_mm):
                                    js = slice(j * mmn, (j + 1) * mmn)
                                    nc.tensor.matmul(
                                        acc_t[:, js],
                                        lhsT=vpp_t[:, k, :],
                                        rhs=ex_t[:, js],
                                        start=(k == 0),
                                        stop=(k == n_k - 1),
                                    )

                        # software pipeline: MM1/ACT run one k-step ahead of
                        # MM2 so PE's in-order stream never starves ACT
                        prev = None
                        for k in range(n_k):
                            exs = emit_mm1_act(k)
                            if prev is not None and mode in ("full", "noepi", "mm12"):
                                emit_mm2(k - 1, *prev)
                            prev = exs
                        if mode in ("full", "noepi", "mm12"):
                            emit_mm2(n_k - 1, *prev)
                        if mode != "full":
                            continue
                        epilogue(acc0, h0, q0)
                        epilogue(acc1, h1, q0)

            if reps == 1:
                emit_body()
            else:
                with tc.For_i(0, reps, 1):
                    emit_body()
    nc.compile()
    return nc


def _shard_inputs(Q, K, V):
    """Full [B,H,S,D] inputs -> per-core in_maps: pair-stacked transposed Q/K
    and ones-augmented, DMA-friendly V layout."""
    bh = _B * _H
    n_k = _S // _KTILE
    # [bh, S, D] -> [bh, D, S] -> [bh//2, 2*D, S] (head pairs stacked)
    Qf = np.ascontiguousarray(
        np.asarray(Q, dtype=np.float32)
        .reshape(bh, _S, _D)
        .transpose(0, 2, 1)
        .reshape(bh // 2, 2 * _D, _S)
    )
    Kf = np.ascontiguousarray(
        np.asarray(K, dtype=np.float32)
        .reshape(bh, _S, _D)
        .transpose(0, 2, 1)
        .reshape(bh // 2, 2 * _D, _S)
    )
    Vf = np.asarray(V, dtype=np.float32).reshape(bh, _S, _D)
    Vf = np.concatenate([Vf, np.ones((bh, _S, 1), np.float32)], axis=2)
    # [bh, S, D+1] -> [bh, k%128, k//128, D+1]
    Vf = np.ascontiguousarray(
        Vf.reshape(bh, n_k, _KTILE, _D + 1).transpose(0, 2, 1, 3)
    )
    hpc2 = _HPC // 2
    in_maps = []
    for c in range(_NCORES):
        in_maps.append(
            {
                "QT": Qf[c * hpc2 : (c + 1) * hpc2],
                "KT": Kf[c * hpc2 : (c + 1) * hpc2],
                "V": Vf[c * _HPC : (c + 1) * _HPC],
            }
        )
    return in_maps


def _unshard_output(results):
    ot = np.concatenate([r["OT"] for r in results], axis=0)  # [32, 64, 2048]
    return np.ascontiguousarray(
        ot.transpose(0, 2, 1).reshape(_B, _H, _S, _D).astype(np.float32)
    )


def kernel(Q, K, V):
    global _nc_cache
    from concourse import bass_utils

    if _nc_cache is None:
        _nc_cache = _build_nc()
    in_maps = _shard_inputs(Q, K, V)
    res = bass_utils.run_bass_kernel_spmd(
        _nc_cache, in_maps, core_ids=list(range(_NCORES))
    )
    return _unshard_output(res.results)
